# revision 8
# baseline (speedup 1.0000x reference)
"""Attention4D kernel for 8 trn2 NeuronCores (Bass/Tile).

Data-parallel over batch (16 items/core). All matmuls bf16 with f32 PSUM.
Talking-heads are Kronecker-structured 128x128 stationaries over
(head, token-subtile)-interleaved attention tiles; softmax normalization is
folded into the th2 moving operand (kron2 * 1/rowsum); the 3x3 depthwise
conv runs as 9 diagonal-stationary matmuls accumulating into the output
PSUM from a zero-padded v layout; biases are exact via per-partition
activation biases and a tau-fold of v_b through the softmax row-sums.

Self-contained: device kernel body + host precompute + cached PJRT runner.
"""

import numpy as np
from contextlib import ExitStack

import concourse.bass as bass
import concourse.tile as tile
from concourse import mybir
from concourse._compat import with_exitstack

F32 = mybir.dt.float32
BF16 = mybir.dt.bfloat16
AF = mybir.ActivationFunctionType
ALU = mybir.AluOpType

B, DIM, RES = 128, 384, 16
NH, KD, D = 8, 32, 128
NHKD, DH = NH * KD, NH * D
N = RES * RES
SCALE = KD ** -0.5
NCORES = 8
NB = B // NCORES  # batch per core

TAPS = [(di, dj) for di in (-1, 0, 1) for dj in (-1, 0, 1)]
CORR_TAPS = [(di, dj) for (di, dj) in TAPS if dj != 0]


# ----------------------------------------------------------------- host side

def host_consts(a):
    """a: dict of np arrays (original reference inputs). Returns const arrays."""
    f32 = np.float32
    q_w = a["q_w"].astype(f32) * f32(SCALE)
    k_w = a["k_w"].astype(f32)
    qk_cat = np.concatenate([q_w.T, k_w.T], axis=1)          # [384, 512]
    wqk = qk_cat.reshape(3, 128, 512).transpose(1, 0, 2).reshape(128, 1536)

    wv = a["v_w"].astype(f32).T.reshape(3, 128, 1024)        # [384,1024]
    wv = wv.transpose(1, 0, 2).reshape(128, 3072)

    wproj = a["proj_w"].astype(f32).T.reshape(8, 128, 384)   # [1024,384]
    wproj = wproj.transpose(1, 0, 2).reshape(128, 3072)

    th1_w, th2_w = a["th1_w"].astype(f32), a["th2_w"].astype(f32)
    kron1 = np.zeros((128, 128), f32)
    kron2 = np.zeros((128, 128), f32)
    for g in range(8):
        for h in range(8):
            for n1 in range(16):
                kron1[h * 16 + n1, g * 16 + n1] = th1_w[g, h]
                kron2[h * 16 + n1, g * 16 + n1] = th2_w[g, h]

    bias_full = a["attn_bias"].astype(f32)[:, a["bias_idxs"]]      # [8,256,256]
    bias2 = np.einsum("gh,hnm->gnm", th1_w, bias_full) \
        + a["th1_b"].astype(f32)[:, None, None]
    expb2 = np.exp(bias2)                                          # [8,256,256]
    # tile layout [g*16+n1, t*256+m]
    expb2_t = expb2.reshape(8, 16, 16, 256).transpose(0, 2, 1, 3)  # g,n1,t,m
    expb2_t = expb2_t.reshape(128, 4096)

    vl_w = a["vl_w"].astype(f32)[:, 0]                             # [1024,3,3]
    dtap = np.zeros((128, 9216), f32)
    for g in range(8):
        for ti, (di, dj) in enumerate(TAPS):
            w = vl_w[g * 128:(g + 1) * 128, di + 1, dj + 1]
            col0 = (g * 9 + ti) * 128
            dtap[np.arange(128), col0 + np.arange(128)] = w

    qkb = np.stack([
        (a["q_b"].astype(f32) * SCALE)[0:128], (a["q_b"].astype(f32) * SCALE)[128:256],
        a["k_b"].astype(f32)[0:128], a["k_b"].astype(f32)[128:256]], axis=1)  # [128,4]
    vb = a["v_b"].astype(f32).reshape(8, 128).T                    # [128,8]
    th2_b = a["th2_b"].astype(f32)
    if not np.allclose(th2_b, 0.0):
        raise NotImplementedError("nonzero th2_b unsupported")
    tau = th2_w.sum(axis=1) + N * th2_b                            # [8]
    obias = a["vl_b"].astype(f32).reshape(8, 128).T + \
        a["v_b"].astype(f32).reshape(8, 128).T * tau[None, :]      # [128,8]
    projb = a["proj_b"].astype(f32).reshape(3, 128).T              # [128,3]

    import ml_dtypes
    bf = lambda x: np.ascontiguousarray(x, dtype=ml_dtypes.bfloat16)
    return {
        "wqk": bf(wqk), "wv": bf(wv), "wproj": bf(wproj),
        "kron1": bf(kron1), "kron2": bf(kron2), "expb2": bf(expb2_t),
        "dtap": bf(dtap), "qkb": np.ascontiguousarray(qkb, f32),
        "vb": np.ascontiguousarray(vb, f32),
        "obias": np.ascontiguousarray(obias, f32),
        "projb": np.ascontiguousarray(projb, f32),
    }


def host_pack_x(x, core):
    """x: [B, 384, 16, 16] f32 -> [NB, 3, 128, 256] bf16 for one core."""
    import ml_dtypes
    xs = x[core * NB:(core + 1) * NB].reshape(NB, 3, 128, N)
    return np.ascontiguousarray(xs, dtype=ml_dtypes.bfloat16)


def host_unpack_out(o):
    """o: [NB, 3, 128, 256] f32 -> [NB, 384, 16, 16]"""
    return o.reshape(NB, DIM, RES, RES)


# --------------------------------------------------------------- device side

CONST_SPECS = [
    ("wqk", (128, 1536), BF16), ("wv", (128, 3072), BF16),
    ("wproj", (128, 3072), BF16), ("kron1", (128, 128), BF16),
    ("kron2", (128, 128), BF16), ("expb2", (128, 4096), BF16),
    ("dtap", (128, 9216), BF16), ("qkb", (128, 4), F32),
    ("vb", (128, 8), F32), ("obias", (128, 8), F32), ("projb", (128, 3), F32),
]


@with_exitstack
def emit(ctx: ExitStack, tc: tile.TileContext, out_d, in_d: dict, nb=NB):
    import os
    STOP = int(os.environ.get("KSTOP", "99"))
    nc = tc.nc
    cp = ctx.enter_context(tc.tile_pool(name="consts", bufs=1))
    c = {}
    for name, shape, dt in CONST_SPECS:
        t = cp.tile(list(shape), dt, tag=name, name=name)
        nc.sync.dma_start(t[:], in_d[name][:])
        c[name] = t

    # persistent blockdiag buffers (zeroed once; diagonals rewritten per batch)
    bdq = [cp.tile([128, 4096], BF16, tag=f"bdq{i}", name=f"bdq{i}") for i in range(2)]
    vpad = [cp.tile([128, 2304], BF16, tag=f"vpad{i}", name=f"vpad{i}") for i in range(2)]
    for t in bdq + vpad:
        nc.vector.memset(t[:], 0.0)

    sb = ctx.enter_context(tc.tile_pool(name="sb", bufs=2))
    ps = ctx.enter_context(tc.tile_pool(name="ps", bufs=3, space="PSUM"))
    pb = ctx.enter_context(tc.tile_pool(name="pb", bufs=2, space="PSUM"))
    pc = ctx.enter_context(tc.tile_pool(name="pc", bufs=3, space="PSUM"))

    x_d = in_d["x"]

    for b in range(nb):
        x_sb = sb.tile([128, 768], BF16, tag="x", bufs=4)
        nc.sync.dma_start(x_sb[:].rearrange("p (c n) -> p c n", c=3),
                          x_d[b].transpose([1, 0, 2]))

        # ---- q, k projections -> q_sb / k_sb [128, 512]
        q_sb = sb.tile([128, 512], BF16, tag="q", bufs=4)
        k_sb = sb.tile([128, 512], BF16, tag="k", bufs=4)
        for op_ in range(2):
            p_qk = ps.tile([128, 512], F32, tag="ps")
            for half in range(2):
                oi = op_ * 2 + half
                for ci in range(3):
                    nc.tensor.matmul(
                        p_qk[:, half * 256:(half + 1) * 256],
                        lhsT=c["wqk"][:, ci * 512 + oi * 128:ci * 512 + (oi + 1) * 128],
                        rhs=x_sb[:, ci * 256:(ci + 1) * 256],
                        start=(ci == 0), stop=(ci == 2))
            dst = (q_sb if op_ == 0 else k_sb)
            nc.scalar.activation(dst[:, 0:256], p_qk[:, 0:256], AF.Identity,
                                 bias=c["qkb"][:, op_ * 2:op_ * 2 + 1])
            nc.scalar.activation(dst[:, 256:512], p_qk[:, 256:512], AF.Identity,
                                 bias=c["qkb"][:, op_ * 2 + 1:op_ * 2 + 2])

        if STOP < 2:
            out_sb = sb.tile([128, 768], F32, tag="out", bufs=4)
            nc.vector.tensor_copy(out_sb[:], x_sb[:])
            nc.sync.dma_start(out_d[b].transpose([1, 0, 2]),
                              out_sb[:].rearrange("p (c n) -> p c n", c=3))
            continue
        # ---- blockdiag build (8 SBUF->SBUF DMAs)
        bd = bdq[b % 2]
        for grp in range(2):
            for hl in range(4):
                src = q_sb[32 * hl:32 * (hl + 1), grp * 256:(grp + 1) * 256]
                src = src.rearrange("p (t n) -> p t n", t=16)
                dst = bd[32 * hl:32 * (hl + 1), grp * 2048:(grp + 1) * 2048]
                dst = dst.rearrange("p (t c) -> p t c", t=16)
                h = grp * 4 + hl
                nc.sync.dma_start(dst[:, :, h * 16:(h + 1) * 16], src)

        if STOP < 3:
            out_sb = sb.tile([128, 768], F32, tag="out", bufs=4)
            nc.vector.tensor_copy(out_sb[:], x_sb[:])
            nc.sync.dma_start(out_d[b].transpose([1, 0, 2]),
                              out_sb[:].rearrange("p (c n) -> p c n", c=3))
            continue
        # ---- vT projection (x stationary) -> vt_sb[2]
        vt_sb = []
        for ni in range(2):
            vt = sb.tile([128, 1024], BF16, tag=f"vt{ni}", bufs=4, name=f"vt{ni}")
            for half in range(2):
                p_vt = ps.tile([128, 512], F32, tag="ps")
                for ci in range(3):
                    nc.tensor.matmul(
                        p_vt[:],
                        lhsT=x_sb[:, ci * 256 + ni * 128:ci * 256 + (ni + 1) * 128],
                        rhs=c["wv"][:, ci * 1024 + half * 512:ci * 1024 + (half + 1) * 512],
                        start=(ci == 0), stop=(ci == 2))
                nc.scalar.activation(vt[:, half * 512:(half + 1) * 512], p_vt[:],
                                     AF.Identity)
            vt_sb.append(vt)

        if STOP < 4:
            out_sb = sb.tile([128, 768], F32, tag="out", bufs=4)
            nc.vector.tensor_copy(out_sb[:], x_sb[:])
            nc.sync.dma_start(out_d[b].transpose([1, 0, 2]),
                              out_sb[:].rearrange("p (c n) -> p c n", c=3))
            continue
        # ---- v projection (padded 16x18 layout, +v_b) -> vpad
        v_pd = vpad[b % 2]
        for op_ in range(4):
            p_v = ps.tile([128, 512], F32, tag="ps")
            for half in range(2):
                oi = op_ * 2 + half
                for ci in range(3):
                    nc.tensor.matmul(
                        p_v[:, half * 256:(half + 1) * 256],
                        lhsT=c["wv"][:, ci * 1024 + oi * 128:ci * 1024 + (oi + 1) * 128],
                        rhs=x_sb[:, ci * 256:(ci + 1) * 256],
                        start=(ci == 0), stop=(ci == 2))
            for half in range(2):
                oi = op_ * 2 + half
                vdst = v_pd[:, oi * 288:(oi + 1) * 288].rearrange(
                    "p (i j) -> p i j", i=16)[:, :, 1:17]
                nc.scalar.activation(
                    vdst,
                    p_v[:, half * 256:(half + 1) * 256].rearrange(
                        "p (i j) -> p i j", i=16),
                    AF.Identity, bias=c["vb"][:, oi:oi + 1])

        if STOP < 5:
            out_sb = sb.tile([128, 768], F32, tag="out", bufs=4)
            nc.vector.tensor_copy(out_sb[:], x_sb[:])
            nc.sync.dma_start(out_d[b].transpose([1, 0, 2]),
                              out_sb[:].rearrange("p (c n) -> p c n", c=3))
            continue
        # ---- S = blockdiag(q)^T k  -> s_sb [(h,n1), (t,m)]
        s_sb = sb.tile([128, 4096], BF16, tag="s")
        for tp in range(8):  # pairs of tiles per psum bank
            p_s = ps.tile([128, 512], F32, tag="ps")
            for t in (2 * tp, 2 * tp + 1):
                o = (t % 2) * 256
                for grp in range(2):
                    nc.tensor.matmul(
                        p_s[:, o:o + 256],
                        lhsT=bd[:, grp * 2048 + t * 128:grp * 2048 + (t + 1) * 128],
                        rhs=k_sb[:, grp * 256:(grp + 1) * 256],
                        start=(grp == 0), stop=(grp == 1))
            if tp % 2 == 0:
                nc.vector.tensor_copy(s_sb[:, tp * 512:(tp + 1) * 512], p_s[:])
            else:
                nc.scalar.activation(s_sb[:, tp * 512:(tp + 1) * 512], p_s[:],
                                     AF.Identity)

        if STOP < 6:
            out_sb = sb.tile([128, 768], F32, tag="out", bufs=4)
            nc.vector.tensor_copy(out_sb[:], x_sb[:])
            nc.sync.dma_start(out_d[b].transpose([1, 0, 2]),
                              out_sb[:].rearrange("p (c n) -> p c n", c=3))
            continue
        # ---- th1 (kron) + exp -> t_sb ; TTR with expb2 -> e_sb + row sums
        t_sb = sb.tile([128, 4096], BF16, tag="texp")
        e_sb = sb.tile([128, 4096], BF16, tag="e")
        esum = sb.tile([128, 16], F32, tag="esum", bufs=4)
        for q4 in range(8):  # pairs: 2 tiles each
            p_t1 = pb.tile([128, 512], F32, tag="pb")
            for ti in range(2):
                t = q4 * 2 + ti
                nc.tensor.matmul(p_t1[:, ti * 256:(ti + 1) * 256],
                                 lhsT=c["kron1"][:], rhs=s_sb[:, t * 256:(t + 1) * 256],
                                 start=True, stop=True)
            nc.scalar.activation(e_sb[:, q4 * 512:(q4 + 1) * 512], p_t1[:], AF.Exp)
            for ti in range(2):
                t = q4 * 2 + ti
                nc.vector.scalar_tensor_tensor(
                    out=e_sb[:, t * 256:(t + 1) * 256],
                    in0=e_sb[:, t * 256:(t + 1) * 256],
                    scalar=1.0,
                    in1=c["expb2"][:, t * 256:(t + 1) * 256],
                    op0=ALU.mult, op1=ALU.mult,
                    accum_out=esum[:, t:t + 1])

        if STOP < 7:
            out_sb = sb.tile([128, 768], F32, tag="out", bufs=4)
            nc.vector.tensor_copy(out_sb[:], x_sb[:])
            nc.sync.dma_start(out_d[b].transpose([1, 0, 2]),
                              out_sb[:].rearrange("p (c n) -> p c n", c=3))
            continue
        # ---- r = 1/rowsum ; kron2 * r
        r_sb = sb.tile([128, 16], F32, tag="r", bufs=4)
        nc.vector.reciprocal(r_sb[:], esum[:])
        kr_sb = sb.tile([128, 2048], BF16, tag="kr", bufs=4)
        for t in range(16):
            nc.vector.tensor_scalar(
                out=kr_sb[:, t * 128:(t + 1) * 128], in0=c["kron2"][:],
                scalar1=r_sb[:, t:t + 1], scalar2=None, op0=ALU.mult)

        if STOP < 8:
            out_sb = sb.tile([128, 768], F32, tag="out", bufs=4)
            nc.vector.tensor_copy(out_sb[:], x_sb[:])
            nc.sync.dma_start(out_d[b].transpose([1, 0, 2]),
                              out_sb[:].rearrange("p (c n) -> p c n", c=3))
            continue
        # ---- th2 + transpose: a2[mc][m, (t,(g,n1))]
        a2_sb = [sb.tile([128, 2048], BF16, tag=f"a2_{mc}", name=f"a2_{mc}") for mc in range(2)]
        for mc in range(2):
            for tg in range(4):
                p_a2 = pc.tile([128, 512], F32, tag="pc")
                for ti in range(4):
                    t = tg * 4 + ti
                    nc.tensor.matmul(
                        p_a2[:, ti * 128:(ti + 1) * 128],
                        lhsT=e_sb[:, t * 256 + mc * 128:t * 256 + (mc + 1) * 128],
                        rhs=kr_sb[:, t * 128:(t + 1) * 128],
                        start=True, stop=True)
                if tg % 2 == 0:
                    nc.vector.tensor_copy(a2_sb[mc][:, tg * 512:(tg + 1) * 512], p_a2[:])
                else:
                    nc.scalar.activation(a2_sb[mc][:, tg * 512:(tg + 1) * 512],
                                         p_a2[:], AF.Identity)

        if STOP < 9:
            out_sb = sb.tile([128, 768], F32, tag="out", bufs=4)
            nc.vector.tensor_copy(out_sb[:], x_sb[:])
            nc.sync.dma_start(out_d[b].transpose([1, 0, 2]),
                              out_sb[:].rearrange("p (c n) -> p c n", c=3))
            continue
        # ---- o = attn2 @ v (+ 9 depthwise taps) ; relu evac
        relu_sb = sb.tile([128, 2048], BF16, tag="relu", bufs=4)
        for g in range(8):
            p_o = pc.tile([128, 256], F32, tag="pc")
            for mc in range(2):
                rhs = a2_sb[mc][:].rearrange("p (t c) -> p t c", t=16)
                nc.tensor.matmul(p_o[:], lhsT=vt_sb[mc][:, g * 128:(g + 1) * 128],
                                 rhs=rhs[:, :, g * 16:(g + 1) * 16],
                                 start=(mc == 0), stop=False)
            # depthwise 3x3 from padded v: one matmul per tap
            vv = v_pd[:, g * 288:(g + 1) * 288].rearrange("p (i j) -> p i j", i=16, j=18)
            taps = TAPS if os.environ.get("KNOTAPS") != "1" else TAPS[4:5]
            for ti, (di, dj) in enumerate(taps):
                i0, ic = max(0, -di), 16 - abs(di)
                nc.tensor.matmul(
                    p_o[:, i0 * 16:(i0 + ic) * 16],
                    lhsT=c["dtap"][:, (g * 9 + ti) * 128:(g * 9 + ti + 1) * 128],
                    rhs=vv[:, i0 + di:i0 + di + ic, dj + 1:dj + 17],
                    start=False, stop=(ti == len(taps) - 1))
            nc.scalar.activation(relu_sb[:, g * 256:(g + 1) * 256], p_o[:],
                                 AF.Relu, bias=c["obias"][:, g:g + 1])

        if STOP < 10:
            out_sb = sb.tile([128, 768], F32, tag="out", bufs=4)
            nc.vector.tensor_copy(out_sb[:], x_sb[:])
            nc.sync.dma_start(out_d[b].transpose([1, 0, 2]),
                              out_sb[:].rearrange("p (c n) -> p c n", c=3))
            continue
        # ---- proj + out
        out_sb = sb.tile([128, 768], F32, tag="out", bufs=4)
        for co in range(3):
            p_p = pc.tile([128, 256], F32, tag="pc")
            for kc in range(8):
                nc.tensor.matmul(
                    p_p[:], lhsT=c["wproj"][:, kc * 384 + co * 128:kc * 384 + (co + 1) * 128],
                    rhs=relu_sb[:, kc * 256:(kc + 1) * 256],
                    start=(kc == 0), stop=(kc == 7))
            nc.scalar.activation(out_sb[:, co * 256:(co + 1) * 256], p_p[:],
                                 AF.Identity, bias=c["projb"][:, co:co + 1])
        nc.sync.dma_start(out_d[b].transpose([1, 0, 2]),
                          out_sb[:].rearrange("p (c n) -> p c n", c=3))


_CACHE = {}


def _build(nb):
    import concourse.bacc as bacc
    import concourse.tile as tile
    from concourse import mybir

    nc = bacc.Bacc("TRN2", target_bir_lowering=False, debug=False)
    ins = {}
    for name, shape, dt in CONST_SPECS:
        ins[name] = nc.dram_tensor(name, list(shape), dt, kind="ExternalInput").ap()
    ins["x"] = nc.dram_tensor("x", [nb, 3, 128, 256], mybir.dt.bfloat16,
                              kind="ExternalInput").ap()
    out = nc.dram_tensor("out", [nb, 3, 128, 256], mybir.dt.bfloat16,
                         kind="ExternalOutput").ap()
    with tile.TileContext(nc) as tc:
        emit(tc, out, ins, nb=nb)
    nc.compile()
    return nc


def _make_runner(nc, n_cores):
    """Mirror bass2jax.run_bass_via_pjrt with a persistent jit + device consts."""
    import jax
    from jax.sharding import Mesh, PartitionSpec, NamedSharding
    from jax.experimental.shard_map import shard_map
    from concourse import bass2jax, mybir

    bass2jax.install_neuronx_cc_hook()

    pname = nc.partition_id_tensor.name if nc.partition_id_tensor else None
    in_names, out_names, out_avals, zero_outs = [], [], [], []
    for alloc in nc.m.functions[0].allocations:
        if not isinstance(alloc, mybir.MemoryLocationSet):
            continue
        name = alloc.memorylocations[0].name
        if alloc.kind == "ExternalInput":
            if name != pname:
                in_names.append(name)
        elif alloc.kind == "ExternalOutput":
            npdt = mybir.dt.np(alloc.dtype)
            out_names.append(name)
            out_avals.append(jax.core.ShapedArray(tuple(alloc.tensor_shape), npdt))
            zero_outs.append(np.zeros(tuple(alloc.tensor_shape), npdt))
    n_params = len(in_names)
    all_names = in_names + out_names
    if pname is not None:
        all_names = all_names + [pname]

    def _body(*args):
        operands = list(args)
        if pname is not None:
            operands.append(bass2jax.partition_id_tensor())
        outs = bass2jax._bass_exec_p.bind(
            *operands,
            out_avals=tuple(out_avals),
            in_names=tuple(all_names),
            out_names=tuple(out_names),
            lowering_input_output_aliases=(),
            sim_require_finite=True,
            sim_require_nnan=True,
            nc=nc,
        )
        return tuple(outs)

    devices = jax.devices()[:n_cores]
    mesh = Mesh(np.asarray(devices), ("core",))
    spec = PartitionSpec("core")
    sharded = jax.jit(
        shard_map(_body, mesh=mesh, in_specs=(spec,) * (n_params + len(out_names)),
                  out_specs=(spec,) * len(out_names), check_rep=False),
        donate_argnums=tuple(range(n_params, n_params + len(out_names))),
        keep_unused=True,
    )
    sharding = NamedSharding(mesh, spec)
    return {
        "jit": sharded, "in_names": in_names, "out_names": out_names,
        "zero_outs": zero_outs, "sharding": sharding, "mesh": mesh,
        "n_cores": n_cores,
    }


def _run(runner, in_maps, consts_dev):
    import jax
    n_cores = runner["n_cores"]
    args = []
    for name in runner["in_names"]:
        if name in consts_dev:
            args.append(consts_dev[name])
        else:
            cat = np.concatenate([np.asarray(m[name]) for m in in_maps], axis=0)
            args.append(jax.device_put(cat, runner["sharding"]))
    prev = _CACHE.get("prev_outs")
    if prev is None:
        prev = [jax.device_put(
            np.zeros((n_cores * z.shape[0], *z.shape[1:]), z.dtype),
            runner["sharding"]) for z in runner["zero_outs"]]
    outs = list(runner["jit"](*args, *prev))
    # keep a copy on device to donate next call (kernel writes every element,
    # so stale values are never observable)
    _CACHE["prev_outs"] = [o.copy() for o in outs]
    return outs


def kernel(**inputs):
    import jax

    a = {k: np.asarray(v) for k, v in inputs.items()}
    hkey = tuple(id(inputs[k]) for k in ("q_w", "k_w", "v_w", "proj_w", "attn_bias"))
    if _CACHE.get("hkey") != hkey:
        _CACHE["consts"] = host_consts(a)
        _CACHE["hkey"] = hkey
        _CACHE.pop("consts_key", None)
    consts = _CACHE["consts"]
    x = np.ascontiguousarray(a["x"], dtype=np.float32)
    nb = NB
    n_cores = NCORES

    if "nc" not in _CACHE:
        _CACHE["nc"] = _build(nb)
        _CACHE["runner"] = _make_runner(_CACHE["nc"], n_cores)
    runner = _CACHE["runner"]

    # weight-derived constants: replicate per core, keep device-resident
    key = id(inputs.get("q_w", None))
    if _CACHE.get("consts_key") != key:
        consts_dev = {}
        for name in runner["in_names"]:
            if name == "x":
                continue
            cat = np.concatenate([consts[name]] * n_cores, axis=0)
            consts_dev[name] = jax.device_put(cat, runner["sharding"])
        _CACHE["consts_dev"] = consts_dev
        _CACHE["consts_key"] = key
    consts_dev = _CACHE["consts_dev"]

    in_maps = [{"x": host_pack_x(x, core)} for core in range(n_cores)]
    outs = _run(runner, in_maps, consts_dev)
    o = np.asarray(outs[0]).astype(np.float32).reshape(n_cores, nb, 3, 128, 256)
    out = np.empty((B, DIM, RES, RES), np.float32)
    for core in range(n_cores):
        out[core * nb:(core + 1) * nb] = host_unpack_out(o[core])
    return out
